# revision 14
# baseline (speedup 1.0000x reference)
"""Trainium2 Bass kernel for nn_CrossAttention (B=4, LQ=4096, S=4096, D=512).

Sharding: data-parallel over (batch, query-half): core = 2*b + half.
Each core computes cross-attention for one batch element and 2048 query rows.
K/V projections are recomputed by both cores of a pair.

Host-side marshalling pre-casts all matmul operands to bf16 and pre-transposes
query/target into the [feature, token] layout the PE needs (bit-identical to
the on-device DVE cast + PE transpose it replaces, but free on the device).
All heavy matmuls run bf16 with fp32 PSUM accumulation. Softmax skips the
max-subtraction (scores bounded ~ +-30, exp stays in fp32 range) and the
normalization is deferred to the output-projection epilogue via per-row
reciprocals. The score->exp->context chain is software-pipelined one s-chunk
deep so the PE never waits on the Exp activation, and each query block's
epilogue (row-sum collapse, output projection, store) is drained inside the
next block's score loop. The final query block is tapered (384/128) so the
last epilogue has little work after the last context matmul.
"""

import numpy as np
import ml_dtypes

B, LQ, S = 4, 4096, 4096
D = 512          # SRC == TGT == 512
P = 128
LQH = LQ // 2    # 2048 query rows per core
DC = D // P      # 4 chunks of the feature dims
SC = S // P      # 32 s-chunks of 128
IB = 512         # kv block width for the K/V projection loop
KB = S // IB     # 8 kv blocks
WS = [512, 512, 512, 384, 128]   # stage C query-block widths (sum = LQH)

_CACHED = {}


def _build_program():
    import concourse.bass as bass
    import concourse.mybir as mybir
    import concourse.tile as tile
    from concourse import bacc
    from concourse.masks import make_identity
    from contextlib import ExitStack

    f32 = mybir.dt.float32
    bf16 = mybir.dt.bfloat16
    AF = mybir.ActivationFunctionType
    OP = mybir.AluOpType

    nc = bacc.Bacc("TRN2", target_bir_lowering=False, debug=False, num_devices=8)

    qT_d = nc.dram_tensor("qT", [D, LQH], bf16, kind="ExternalInput").ap()
    tgtT_d = nc.dram_tensor("tgtT", [D, S], bf16, kind="ExternalInput").ap()
    w_d = {}
    b_d = {}
    for nm in ("wq", "wk", "wv", "wo"):
        w_d[nm] = nc.dram_tensor(nm, [D, D], bf16, kind="ExternalInput").ap()
    for nm in ("bq", "bk", "bv", "bo"):
        b_d[nm] = nc.dram_tensor(nm, [D], f32, kind="ExternalInput").ap()
    out_dram = nc.dram_tensor("out", [LQH, D], f32, kind="ExternalOutput").ap()

    with tile.TileContext(nc) as tc, ExitStack() as ctx:
        const = ctx.enter_context(tc.tile_pool(name="const", bufs=1))
        big = ctx.enter_context(tc.tile_pool(name="big", bufs=1))
        ptp = ctx.enter_context(tc.tile_pool(name="ptp", bufs=6))
        ctxp = ctx.enter_context(tc.tile_pool(name="ctxp", bufs=2))
        outp = ctx.enter_context(tc.tile_pool(name="outp", bufs=4))
        smallp = ctx.enter_context(tc.tile_pool(name="smallp", bufs=2))
        ps_pt = ctx.enter_context(tc.tile_pool(name="ps_pt", bufs=4, space="PSUM"))
        ps_ctx = ctx.enter_context(tc.tile_pool(name="ps_ctx", bufs=4, space="PSUM"))

        # ---- tiny constants first (identity keeps the PE busy at t=0) ----
        ident_f = const.tile([P, P], f32, tag="ident_f", name="ident_f")
        make_identity(nc, ident_f)
        ones_col = const.tile([P, 1], f32, tag="ones_col", name="ones_col")
        nc.vector.memset(ones_col, 1.0)

        b_col = {}
        for nm in ("bq", "bk"):
            bc = const.tile([P, DC], f32, tag=f"col_{nm}", name=f"{nm}_col")
            nc.gpsimd.dma_start(out=bc, in_=b_d[nm].rearrange("(c p) -> p c", p=P))
            b_col[nm] = bc
        b_rep = {}
        for nm in ("bv", "bo"):
            br = const.tile([P, D], f32, tag=f"rep_{nm}", name=f"{nm}_rep")
            src = b_d[nm]
            bcast = bass.AP(tensor=src.tensor, offset=src.offset,
                            ap=[[0, P]] + list(src.ap))
            nc.gpsimd.dma_start(out=br, in_=bcast)
            b_rep[nm] = br

        # staging row for the reciprocal transpose: row 0 live, rows 1.. zero
        rstage = const.tile([P, IB], f32, tag="rstage", name="rstage")
        nc.vector.memset(rstage, 0.0)

        # ---- big DMAs: one fast (sync HWDGE) queue, strictly in order of
        # first use, with the first-needed tensors split fine-grained so the
        # PE starts as early as possible ----
        w_t = {}

        def load_weight(nm, chunks=1):
            wt = const.tile([P, DC, D], bf16, tag=f"w_{nm}", name=f"{nm}_bf")
            src = w_d[nm].rearrange("(c p) n -> p c n", p=P)
            cw = D // chunks
            for c in range(chunks):
                nc.sync.dma_start(wt[:, :, c * cw:(c + 1) * cw],
                                  src[:, :, c * cw:(c + 1) * cw])
            w_t[nm] = wt

        # Inputs land in per-granule TILES (not slices of one tile): the dep
        # tracker is whole-tile for DMA writes, so one big tile would make
        # the first consumer wait for every chunk.
        QG = 256                       # query DMA/projection granule
        NQG = LQH // QG
        qT_r = qT_d.rearrange("(c p) i -> p c i", p=P)
        tgtT_r = tgtT_d.rearrange("(c p) s -> p c s", p=P)
        qin_g = [big.tile([P, DC, QG], bf16, tag=f"qin{g}", name=f"qin{g}")
                 for g in range(NQG)]
        tgt_b = [big.tile([P, DC, IB], bf16, tag=f"tgt{sb}", name=f"tgt{sb}")
                 for sb in range(KB)]

        def load_qin(g):
            nc.sync.dma_start(qin_g[g], qT_r[:, :, g * QG:(g + 1) * QG])

        def load_tgt(sb):
            nc.sync.dma_start(tgt_b[sb], tgtT_r[:, :, sb * IB:(sb + 1) * IB])

        load_weight("wq")
        load_qin(0)
        load_qin(1)
        load_weight("wk")
        load_qin(2)
        load_qin(3)
        load_tgt(0)
        load_weight("wv")
        load_tgt(1)
        for g in range(4, NQG):
            load_qin(g)
        for sb in range(2, KB):
            load_tgt(sb)
        load_weight("wo")

        # ---- stage A: query projection -> qTp = [dout, i] ----
        qTp = big.tile([P, DC, LQH], bf16, tag="qTp", name="qTp")
        for g in range(NQG):
            for tcc in range(DC):
                pool = ps_pt if (g * DC + tcc) % 2 else ps_ctx
                tagn = "ps_pt" if (g * DC + tcc) % 2 else "ps_ctx"
                ps = pool.tile([P, QG], f32, tag=tagn, name=f"psq_{g}_{tcc}")
                for dc in range(DC):
                    nc.tensor.matmul(ps, w_t["wq"][:, dc, tcc * P:(tcc + 1) * P],
                                     qin_g[g][:, dc, :],
                                     start=(dc == 0), stop=(dc == DC - 1))
                nc.vector.tensor_tensor(qTp[:, tcc, g * QG:(g + 1) * QG], ps,
                                        b_col["bq"][:, tcc:tcc + 1].to_broadcast([P, QG]),
                                        OP.add)

        # ---- stage B: K/V projections ----
        kT = big.tile([P, DC, S], bf16, tag="kT", name="kT")
        vv = big.tile([P, SC, D], bf16, tag="vv", name="vv")
        for sb in range(KB):
            ssl = slice(sb * IB, (sb + 1) * IB)
            # K psums rotate through ps_ctx (idle until stage C), V through
            # ps_pt: 8 banks of slack so the PE never waits on the DVE drain
            for tcc in range(DC):
                ps = ps_ctx.tile([P, IB], f32, tag="ps_ctx", name=f"psk_{sb}_{tcc}")
                for dc in range(DC):
                    nc.tensor.matmul(ps, w_t["wk"][:, dc, tcc * P:(tcc + 1) * P],
                                     tgt_b[sb][:, dc, :],
                                     start=(dc == 0), stop=(dc == DC - 1))
                nc.vector.tensor_tensor(kT[:, tcc, ssl], ps,
                                        b_col["bk"][:, tcc:tcc + 1].to_broadcast([P, IB]),
                                        OP.add)
            for sl in range(IB // P):
                scc = sb * (IB // P) + sl
                ps = ps_pt.tile([P, D], f32, tag="ps_pt", name=f"psv_{sb}_{sl}")
                for dc in range(DC):
                    nc.tensor.matmul(ps, tgt_b[sb][:, dc, sl * P:(sl + 1) * P],
                                     w_t["wv"][:, dc, :],
                                     start=(dc == 0), stop=(dc == DC - 1))
                nc.vector.tensor_tensor(vv[:, scc, :], ps, b_rep["bv"], OP.add)

        # ---- stage C: attention + output projection ----
        def make_epilogue(ib, W, off, ctxT, acc):
            """Yields epilogue steps for block ib; drained inside the next
            block's score loop so the PE work here hides in its stream."""
            NCH = W // P
            rs_ps = ps_pt.tile([1, W], f32, tag="ps_pt", name=f"rs_{ib}")
            nc.tensor.matmul(rs_ps, ones_col, acc, start=True, stop=True)
            nc.vector.tensor_copy(out=rstage[0:1, :W], in_=rs_ps)
            yield
            rt_ps = ps_pt.tile([P, W], f32, tag="ps_pt", name=f"rt_{ib}")
            for ic in range(NCH):
                nc.tensor.transpose(rt_ps[:, ic * P:(ic + 1) * P],
                                    rstage[:, ic * P:(ic + 1) * P], ident_f)
            rsum_col = smallp.tile([P, NCH], f32, tag="rsum_col", name=f"rsc_{ib}")
            nc.scalar.activation(rsum_col,
                                 rt_ps.rearrange("p (c q) -> p c q", c=NCH)[:, :, 0],
                                 AF.Copy)
            rc_sb = smallp.tile([P, NCH], f32, tag="rc_sb", name=f"rc_{ib}")
            nc.vector.reciprocal(rc_sb, rsum_col)
            yield
            # the very last chunk of the kernel streams out in halves so the
            # post-matmul scale/bias/store chain is half as long
            halves = 2 if ib == len(WS) - 1 else 1
            for ic in range(NCH):
                op_ps = ps_pt.tile([P, D], f32, tag="ps_pt", name=f"op_{ib}_{ic}")
                for dpc in range(DC):
                    nc.tensor.matmul(op_ps, ctxT[:, dpc, ic * P:(ic + 1) * P],
                                     w_t["wo"][:, dpc, :],
                                     start=(dpc == 0), stop=(dpc == DC - 1))
                ot_s = outp.tile([P, D], f32, tag="out_s", name=f"ots_{ib}_{ic}")
                ot = outp.tile([P, D], f32, tag="out_t", name=f"ot_{ib}_{ic}")
                hw = D // halves
                for h in range(halves):
                    hs = slice(h * hw, (h + 1) * hw)
                    nc.scalar.activation(ot_s[:, hs], op_ps[:, hs], AF.Copy,
                                         scale=rc_sb[:, ic:ic + 1])
                    nc.vector.tensor_tensor(ot[:, hs], ot_s[:, hs],
                                            b_rep["bo"][:, hs], OP.add)
                    nc.sync.dma_start(
                        out_dram[off + ic * P: off + (ic + 1) * P, hs], ot[:, hs])
                yield

        offs = [sum(WS[:i]) for i in range(len(WS))]
        prev_epi = None
        for ib, W in enumerate(WS):
            off = offs[ib]
            ctx_ps = [ps_ctx.tile([P, W], f32, tag="ps_ctx", name=f"ctx_{ib}_{d}")
                      for d in range(DC)]
            acc = smallp.tile([P, W], f32, tag="rs_acc", name=f"rsacc_{ib}")

            def emit_ctx(pexp, pscc):
                for dpc in range(DC):
                    nc.tensor.matmul(ctx_ps[dpc], vv[:, pscc, dpc * P:(dpc + 1) * P],
                                     pexp, start=(pscc == 0), stop=(pscc == SC - 1))

            pend = []          # ctx matmuls run 2 s-chunks behind the scores
            for scc in range(SC):
                pt_ps = ps_pt.tile([P, W], f32, tag="ps_pt", name=f"pt_{ib}_{scc}")
                for tcc in range(DC):
                    nc.tensor.matmul(pt_ps, kT[:, tcc, scc * P:(scc + 1) * P],
                                     qTp[:, tcc, off:off + W],
                                     start=(tcc == 0), stop=(tcc == DC - 1))
                pt_exp = ptp.tile([P, W], bf16, tag="pt_exp", name=f"pte_{ib}_{scc}")
                nc.scalar.activation(pt_exp, pt_ps, AF.Exp)
                if scc == 0:
                    nc.vector.tensor_copy(out=acc, in_=pt_exp)
                else:
                    nc.vector.tensor_tensor(acc, acc, pt_exp, OP.add)
                pend.append((pt_exp, scc))
                if len(pend) > 2:
                    emit_ctx(*pend.pop(0))
                if prev_epi is not None and 1 <= scc <= 10:
                    next(prev_epi, None)
            for p in pend:
                emit_ctx(*p)

            # unnormalized context -> bf16 right away (frees the ctx banks)
            ctxT = ctxp.tile([P, DC, W], bf16, tag="ctxT", name=f"ctxT_{ib}")
            for dpc in range(DC):
                nc.vector.tensor_copy(out=ctxT[:, dpc, :], in_=ctx_ps[dpc])

            prev_epi = make_epilogue(ib, W, off, ctxT, acc)

        # drain the last block's epilogue
        for _ in prev_epi:
            pass

    nc.compile()
    return nc


def _get_nc():
    if "nc" not in _CACHED:
        _CACHED["nc"] = _build_program()
    return _CACHED["nc"]


def _make_in_maps(query, target, wq, bq, wk, bk, wv, bv, wo, bo):
    bf = ml_dtypes.bfloat16
    query = np.asarray(query, dtype=np.float32)
    target = np.asarray(target, dtype=np.float32)
    consts = {
        "wq": np.asarray(wq, np.float32).astype(bf),
        "wk": np.asarray(wk, np.float32).astype(bf),
        "wv": np.asarray(wv, np.float32).astype(bf),
        "wo": np.asarray(wo, np.float32).astype(bf),
        "bq": np.asarray(bq, np.float32), "bk": np.asarray(bk, np.float32),
        "bv": np.asarray(bv, np.float32), "bo": np.asarray(bo, np.float32),
    }
    in_maps = []
    for core in range(8):
        b, h = divmod(core, 2)
        qh = query[b, h * LQH:(h + 1) * LQH].astype(bf)     # [LQH, D]
        # faithful to the torch reshape: raw reinterpret of [512, 4096]
        tgt = np.ascontiguousarray(target[b]).reshape(S, D).astype(bf)
        in_maps.append({
            "qT": np.ascontiguousarray(qh.T),               # [D, LQH]
            "tgtT": np.ascontiguousarray(tgt.T),            # [D, S]
            **consts,
        })
    return in_maps


def kernel(query, target, wq, bq, wk, bk, wv, bv, wo, bo):
    from concourse import bass_utils
    nc = _get_nc()
    in_maps = _make_in_maps(query, target, wq, bq, wk, bk, wv, bv, wo, bo)
    res = bass_utils.run_bass_kernel_spmd(nc, in_maps, core_ids=list(range(8)))
    out = np.empty((B, LQ, D), np.float32)
    for core in range(8):
        b, h = divmod(core, 2)
        out[b, h * LQH:(h + 1) * LQH] = res.results[core]["out"]
    return out


# revision 18
# speedup vs baseline: 1.0049x; 1.0049x over previous
"""Trainium2 Bass kernel for nn_CrossAttention (B=4, LQ=4096, S=4096, D=512).

Sharding: data-parallel over (batch, query-half): core = 2*b + half.
Each core computes cross-attention for one batch element and 2048 query rows.
K/V projections are recomputed by both cores of a pair.

Host-side marshalling pre-casts all matmul operands to bf16 and pre-transposes
query/target into the [feature, token] layout the PE needs (bit-identical to
the on-device DVE cast + PE transpose it replaces, but free on the device).
All heavy matmuls run bf16 with fp32 PSUM accumulation. Softmax skips the
max-subtraction (scores bounded ~ +-30, exp stays in fp32 range) and the
normalization is deferred to the output-projection epilogue via per-row
reciprocals. The score->exp->context chain is software-pipelined one s-chunk
deep so the PE never waits on the Exp activation, and each query block's
epilogue (row-sum collapse, output projection, store) is drained inside the
next block's score loop. The final query block is tapered (384/128) so the
last epilogue has little work after the last context matmul.
"""

import numpy as np
import ml_dtypes

B, LQ, S = 4, 4096, 4096
D = 512          # SRC == TGT == 512
P = 128
LQH = LQ // 2    # 2048 query rows per core
DC = D // P      # 4 chunks of the feature dims
SC = S // P      # 32 s-chunks of 128
IB = 512         # kv block width for the K/V projection loop
KB = S // IB     # 8 kv blocks
WS = [512, 512, 512, 384, 128]   # stage C query-block widths (sum = LQH)

_CACHED = {}


def _build_program():
    import concourse.bass as bass
    import concourse.mybir as mybir
    import concourse.tile as tile
    from concourse import bacc
    from concourse.masks import make_identity
    from contextlib import ExitStack

    f32 = mybir.dt.float32
    bf16 = mybir.dt.bfloat16
    AF = mybir.ActivationFunctionType
    OP = mybir.AluOpType

    nc = bacc.Bacc("TRN2", target_bir_lowering=False, debug=False, num_devices=8)

    # All inputs are host-packed into the exact partition-major SBUF layout:
    # every DMA below is a pure [128 x contiguous-bytes] 2D transfer (cheap
    # descriptors, full burst efficiency).
    QG = 256                       # query DMA/projection granule
    NQG = LQH // QG
    qT_d = nc.dram_tensor("qT", [P, NQG * DC * QG], bf16, kind="ExternalInput").ap()
    tgtT_d = nc.dram_tensor("tgtT", [P, KB * DC * IB], bf16, kind="ExternalInput").ap()
    w_d = {}
    b_d = {}
    w_d["wq"] = nc.dram_tensor("wq", [P, DC * DC * P], bf16, kind="ExternalInput").ap()
    for nm in ("wk", "wv", "wo"):
        w_d[nm] = nc.dram_tensor(nm, [P, DC * D], bf16, kind="ExternalInput").ap()
    for nm in ("bq", "bk"):
        b_d[nm] = nc.dram_tensor(nm, [P, DC], f32, kind="ExternalInput").ap()
    for nm in ("bv", "bo"):
        b_d[nm] = nc.dram_tensor(nm, [P, D], f32, kind="ExternalInput").ap()
    out_dram = nc.dram_tensor("out", [LQH, D], f32, kind="ExternalOutput").ap()

    with tile.TileContext(nc) as tc, ExitStack() as ctx:
        const = ctx.enter_context(tc.tile_pool(name="const", bufs=1))
        big = ctx.enter_context(tc.tile_pool(name="big", bufs=1))
        ptp = ctx.enter_context(tc.tile_pool(name="ptp", bufs=6))
        ctxp = ctx.enter_context(tc.tile_pool(name="ctxp", bufs=2))
        outp = ctx.enter_context(tc.tile_pool(name="outp", bufs=4))
        smallp = ctx.enter_context(tc.tile_pool(name="smallp", bufs=2))
        ps_pt = ctx.enter_context(tc.tile_pool(name="ps_pt", bufs=4, space="PSUM"))
        ps_ctx = ctx.enter_context(tc.tile_pool(name="ps_ctx", bufs=4, space="PSUM"))

        # ---- tiny constants first (identity keeps the PE busy at t=0) ----
        ident_f = const.tile([P, P], f32, tag="ident_f", name="ident_f")
        make_identity(nc, ident_f)
        ones_col = const.tile([P, 1], f32, tag="ones_col", name="ones_col")
        nc.vector.memset(ones_col, 1.0)

        b_col = {}
        for nm in ("bq", "bk"):
            bc = const.tile([P, DC], f32, tag=f"col_{nm}", name=f"{nm}_col")
            nc.gpsimd.dma_start(out=bc, in_=b_d[nm])
            b_col[nm] = bc
        b_rep = {}
        for nm in ("bv", "bo"):
            br = const.tile([P, D], f32, tag=f"rep_{nm}", name=f"{nm}_rep")
            nc.gpsimd.dma_start(out=br, in_=b_d[nm])
            b_rep[nm] = br

        # staging row for the reciprocal transpose: row 0 live, rows 1.. zero
        rstage = const.tile([P, IB], f32, tag="rstage", name="rstage")
        nc.vector.memset(rstage, 0.0)

        # ---- big DMAs: one fast (sync HWDGE) queue, strictly in order of
        # first use. Inputs land in per-granule TILES (not slices of one
        # tile): the dep tracker is whole-tile for DMA writes, so one big
        # tile would make the first consumer wait for every chunk. ----
        w_t = {}

        def load_weight(nm):
            wt = const.tile([P, DC, D], bf16, tag=f"w_{nm}", name=f"{nm}_bf")
            nc.sync.dma_start(wt, w_d[nm].rearrange("p (c n) -> p c n", c=DC))
            w_t[nm] = wt

        # wq is tcc-chunked so the very first projection only waits on 128KB
        wq_t = const.tile([P, DC, DC, P], bf16, tag="w_wq", name="wq_bf")
        wq_r = w_d["wq"].rearrange("p (t c n) -> p t c n", t=DC, c=DC)
        nc.sync.dma_start(wq_t[:, 0], wq_r[:, 0])
        qin_g = [big.tile([P, DC, QG], bf16, tag=f"qin{g}", name=f"qin{g}")
                 for g in range(NQG)]
        tgt_b = [big.tile([P, DC, IB], bf16, tag=f"tgt{sb}", name=f"tgt{sb}")
                 for sb in range(KB)]
        qT_r = qT_d.rearrange("p (g c i) -> p g c i", g=NQG, c=DC)
        tgtT_r = tgtT_d.rearrange("p (b c s) -> p b c s", b=KB, c=DC)

        def load_qin(g):
            nc.sync.dma_start(qin_g[g], qT_r[:, g])

        def load_tgt(sb):
            nc.sync.dma_start(tgt_b[sb], tgtT_r[:, sb])

        load_qin(0)
        for t in range(1, DC):
            nc.sync.dma_start(wq_t[:, t], wq_r[:, t])
        load_qin(1)
        load_tgt(0)
        load_weight("wk")
        load_qin(2)
        load_qin(3)
        load_tgt(1)
        load_weight("wv")
        for g in range(4, NQG):
            load_qin(g)
        for sb in range(2, KB):
            load_tgt(sb)
        load_weight("wo")

        # ---- stage A: query projection -> qTp = [dout, i] ----
        qTp = big.tile([P, DC, LQH], bf16, tag="qTp", name="qTp")
        for g in range(NQG):
            for tcc in range(DC):
                pool = ps_pt if (g * DC + tcc) % 2 else ps_ctx
                tagn = "ps_pt" if (g * DC + tcc) % 2 else "ps_ctx"
                ps = pool.tile([P, QG], f32, tag=tagn, name=f"psq_{g}_{tcc}")
                for dc in range(DC):
                    nc.tensor.matmul(ps, wq_t[:, tcc, dc, :],
                                     qin_g[g][:, dc, :],
                                     start=(dc == 0), stop=(dc == DC - 1))
                nc.vector.tensor_tensor(qTp[:, tcc, g * QG:(g + 1) * QG], ps,
                                        b_col["bq"][:, tcc:tcc + 1].to_broadcast([P, QG]),
                                        OP.add)

        # ---- stage B: K/V projections ----
        kT = big.tile([P, DC, S], bf16, tag="kT", name="kT")
        vv = big.tile([P, SC, D], bf16, tag="vv", name="vv")
        for sb in range(KB):
            ssl = slice(sb * IB, (sb + 1) * IB)
            # K psums rotate through ps_ctx (idle until stage C), V through
            # ps_pt: 8 banks of slack so the PE never waits on the DVE drain
            for tcc in range(DC):
                ps = ps_ctx.tile([P, IB], f32, tag="ps_ctx", name=f"psk_{sb}_{tcc}")
                for dc in range(DC):
                    nc.tensor.matmul(ps, w_t["wk"][:, dc, tcc * P:(tcc + 1) * P],
                                     tgt_b[sb][:, dc, :],
                                     start=(dc == 0), stop=(dc == DC - 1))
                nc.vector.tensor_tensor(kT[:, tcc, ssl], ps,
                                        b_col["bk"][:, tcc:tcc + 1].to_broadcast([P, IB]),
                                        OP.add)
            for sl in range(IB // P):
                scc = sb * (IB // P) + sl
                ps = ps_pt.tile([P, D], f32, tag="ps_pt", name=f"psv_{sb}_{sl}")
                for dc in range(DC):
                    nc.tensor.matmul(ps, tgt_b[sb][:, dc, sl * P:(sl + 1) * P],
                                     w_t["wv"][:, dc, :],
                                     start=(dc == 0), stop=(dc == DC - 1))
                nc.vector.tensor_tensor(vv[:, scc, :], ps, b_rep["bv"], OP.add)

        # ---- stage C: attention + output projection ----
        def make_epilogue(ib, W, off, ctxT, acc):
            """Yields epilogue steps for block ib; drained inside the next
            block's score loop so the PE work here hides in its stream."""
            NCH = W // P
            rs_ps = ps_pt.tile([1, W], f32, tag="ps_pt", name=f"rs_{ib}")
            nc.tensor.matmul(rs_ps, ones_col, acc, start=True, stop=True)
            nc.vector.tensor_copy(out=rstage[0:1, :W], in_=rs_ps)
            yield
            rt_ps = ps_pt.tile([P, W], f32, tag="ps_pt", name=f"rt_{ib}")
            for ic in range(NCH):
                nc.tensor.transpose(rt_ps[:, ic * P:(ic + 1) * P],
                                    rstage[:, ic * P:(ic + 1) * P], ident_f)
            rsum_col = smallp.tile([P, NCH], f32, tag="rsum_col", name=f"rsc_{ib}")
            nc.scalar.activation(rsum_col,
                                 rt_ps.rearrange("p (c q) -> p c q", c=NCH)[:, :, 0],
                                 AF.Copy)
            rc_sb = smallp.tile([P, NCH], f32, tag="rc_sb", name=f"rc_{ib}")
            nc.vector.reciprocal(rc_sb, rsum_col)
            yield
            # the very last chunk of the kernel streams out in halves so the
            # post-matmul scale/bias/store chain is half as long
            halves = 2 if ib == len(WS) - 1 else 1
            for ic in range(NCH):
                op_ps = ps_pt.tile([P, D], f32, tag="ps_pt", name=f"op_{ib}_{ic}")
                for dpc in range(DC):
                    nc.tensor.matmul(op_ps, ctxT[:, dpc, ic * P:(ic + 1) * P],
                                     w_t["wo"][:, dpc, :],
                                     start=(dpc == 0), stop=(dpc == DC - 1))
                ot_s = outp.tile([P, D], f32, tag="out_s", name=f"ots_{ib}_{ic}")
                ot = outp.tile([P, D], f32, tag="out_t", name=f"ot_{ib}_{ic}")
                hw = D // halves
                for h in range(halves):
                    hs = slice(h * hw, (h + 1) * hw)
                    nc.scalar.activation(ot_s[:, hs], op_ps[:, hs], AF.Copy,
                                         scale=rc_sb[:, ic:ic + 1])
                    nc.vector.tensor_tensor(ot[:, hs], ot_s[:, hs],
                                            b_rep["bo"][:, hs], OP.add)
                    nc.sync.dma_start(
                        out_dram[off + ic * P: off + (ic + 1) * P, hs], ot[:, hs])
                yield

        offs = [sum(WS[:i]) for i in range(len(WS))]
        prev_epi = None
        for ib, W in enumerate(WS):
            off = offs[ib]
            ctx_ps = [ps_ctx.tile([P, W], f32, tag="ps_ctx", name=f"ctx_{ib}_{d}")
                      for d in range(DC)]
            acc = smallp.tile([P, W], f32, tag="rs_acc", name=f"rsacc_{ib}")

            def emit_ctx(pexp, pscc):
                for dpc in range(DC):
                    nc.tensor.matmul(ctx_ps[dpc], vv[:, pscc, dpc * P:(dpc + 1) * P],
                                     pexp, start=(pscc == 0), stop=(pscc == SC - 1))

            pend = []          # ctx matmuls run 2 s-chunks behind the scores
            for scc in range(SC):
                pt_ps = ps_pt.tile([P, W], f32, tag="ps_pt", name=f"pt_{ib}_{scc}")
                for tcc in range(DC):
                    nc.tensor.matmul(pt_ps, kT[:, tcc, scc * P:(scc + 1) * P],
                                     qTp[:, tcc, off:off + W],
                                     start=(tcc == 0), stop=(tcc == DC - 1))
                pt_exp = ptp.tile([P, W], bf16, tag="pt_exp", name=f"pte_{ib}_{scc}")
                nc.scalar.activation(pt_exp, pt_ps, AF.Exp)
                if scc == 0:
                    nc.vector.tensor_copy(out=acc, in_=pt_exp)
                else:
                    nc.vector.tensor_tensor(acc, acc, pt_exp, OP.add)
                pend.append((pt_exp, scc))
                if len(pend) > 2:
                    emit_ctx(*pend.pop(0))
                if prev_epi is not None and 1 <= scc <= 10:
                    next(prev_epi, None)
            for p in pend:
                emit_ctx(*p)

            # unnormalized context -> bf16 right away (frees the ctx banks)
            ctxT = ctxp.tile([P, DC, W], bf16, tag="ctxT", name=f"ctxT_{ib}")
            for dpc in range(DC):
                nc.vector.tensor_copy(out=ctxT[:, dpc, :], in_=ctx_ps[dpc])

            prev_epi = make_epilogue(ib, W, off, ctxT, acc)

        # drain the last block's epilogue
        for _ in prev_epi:
            pass

    nc.compile()
    return nc


def _get_nc():
    if "nc" not in _CACHED:
        _CACHED["nc"] = _build_program()
    return _CACHED["nc"]


def _make_in_maps(query, target, wq, bq, wk, bk, wv, bv, wo, bo):
    bf = ml_dtypes.bfloat16
    QG = 256
    NQG = LQH // QG
    query = np.asarray(query, dtype=np.float32)
    target = np.asarray(target, dtype=np.float32)

    def pack_tok(xT, nblk, blk):      # [D, N] -> [P, nblk*DC*blk] partition-major
        return np.ascontiguousarray(
            xT.reshape(DC, P, nblk, blk).transpose(1, 2, 0, 3).reshape(P, -1))

    def pack_w(w):                    # [D, D] -> [P, DC*D], din partition-major
        return np.ascontiguousarray(
            np.asarray(w, np.float32).astype(bf)
            .reshape(DC, P, D).transpose(1, 0, 2).reshape(P, -1))

    wq_b = (np.asarray(wq, np.float32).astype(bf)
            .reshape(DC, P, DC, P).transpose(1, 2, 0, 3).reshape(P, -1))
    consts = {
        "wq": np.ascontiguousarray(wq_b),
        "wk": pack_w(wk), "wv": pack_w(wv), "wo": pack_w(wo),
        "bq": np.ascontiguousarray(np.asarray(bq, np.float32).reshape(DC, P).T),
        "bk": np.ascontiguousarray(np.asarray(bk, np.float32).reshape(DC, P).T),
        "bv": np.ascontiguousarray(
            np.broadcast_to(np.asarray(bv, np.float32), (P, D))),
        "bo": np.ascontiguousarray(
            np.broadcast_to(np.asarray(bo, np.float32), (P, D))),
    }
    in_maps = []
    for core in range(8):
        b, h = divmod(core, 2)
        qh = query[b, h * LQH:(h + 1) * LQH].astype(bf)     # [LQH, D]
        # faithful to the torch reshape: raw reinterpret of [512, 4096]
        tgt = np.ascontiguousarray(target[b]).reshape(S, D).astype(bf)
        in_maps.append({
            "qT": pack_tok(qh.T, NQG, QG),
            "tgtT": pack_tok(tgt.T, KB, IB),
            **consts,
        })
    return in_maps


def kernel(query, target, wq, bq, wk, bk, wv, bv, wo, bo):
    from concourse import bass_utils
    nc = _get_nc()
    in_maps = _make_in_maps(query, target, wq, bq, wk, bk, wv, bv, wo, bo)
    res = bass_utils.run_bass_kernel_spmd(nc, in_maps, core_ids=list(range(8)))
    out = np.empty((B, LQ, D), np.float32)
    for core in range(8):
        b, h = divmod(core, 2)
        out[b, h * LQH:(h + 1) * LQH] = res.results[core]["out"]
    return out
